# revision 8
# baseline (speedup 1.0000x reference)
"""MultiHeadCrossAttention on 8 trn2 NeuronCores.

Strategy: pure batch data-parallelism — B=8 batches, one per core, zero
collectives. Each core computes its batch's full cross-attention:

  q = x @ Wq + bq ; k = xa @ Wk ; v = xa @ Wv + bv
  qk[h,t,s] = (q*sc)·(k*sc)   (sc = Dh**-0.25; folded as k *= 1/8 exactly)
  out = softmax(qk) @ v @ Wo + bo

Per-core kernel layout (PE matmul computes out[M,N] = lhsT[K,M].T @ rhs[K,N],
contraction K on partitions):
  - x/xa are PE-transposed once into xT[d,t] / xaT[d,s] (PSUM->SBUF via DMA).
  - per head-pair j: qT[d,t], kT[d,s] column slices; v[s,d] per 4-head group.
  - qk is computed twice: once as [t,s] (streamed straight from PSUM to the
    HBM qk output), once as [s,t] feeding ACT exp -> eT.
  - U^T = [v | 1]^T @ eT gives the unnormalized head output and the softmax
    denominator Z (row 64) in one PSUM accumulation.
  - normalize with 1/Z partition-broadcast, then out = wvT.T @ Wo + bo.

All matmuls run in float32r (TF32-like; 1 cyc/row at N>=256; ~1e-4 rel err).
"""
import sys

import numpy as np

try:
    import concourse.bacc as bacc
except ImportError:  # fresh grading dir: make sure the repo root is importable
    for p in ("/opt/trn_rl_repo", "/root/.axon_site/_ro/trn_rl_repo"):
        if p not in sys.path:
            sys.path.append(p)
    import concourse.bacc as bacc

import concourse.bass as bass
import concourse.mybir as mybir
import concourse.tile as tile
from concourse import bass_utils
from concourse.masks import make_identity

F32 = mybir.dt.float32
F32R = mybir.dt.float32r
AF = mybir.ActivationFunctionType

B, T, S, D, H, Dh = 8, 448, 1500, 1024, 16, 64
ND = D // 128          # 8 d-tiles / k-tiles of 128
TT, NTT = 112, 4       # t tiling: 4 x 112
ST, NST = 125, 12      # s tiling for qkT / v: 12 x 125
SC, NSC = 500, 3       # s chunks for qk(a) / kT: 3 x 500 (4 x 125 each)
KSCALE = 0.125         # (Dh**-0.25)**2 = 1/8, exact in fp


def build():
    nc = bacc.Bacc("TRN2", target_bir_lowering=False, debug=False)
    aps = {}
    for name, shape, dt in [
        ("x", [T, D], F32R), ("xa", [S, D], F32R),
        ("Wq", [D, D], F32R), ("Wk", [D, D], F32R),
        ("Wv", [D, D], F32R), ("Wo", [D, D], F32R),
        ("bq", [D], F32), ("bv", [D], F32), ("bo", [D], F32),
    ]:
        aps[name] = nc.dram_tensor(name, shape, dt, kind="ExternalInput").ap()
    aps["out"] = nc.dram_tensor("out", [T, D], F32, kind="ExternalOutput").ap()
    aps["qk"] = nc.dram_tensor("qk", [H, T, S], F32, kind="ExternalOutput").ap()

    with tile.TileContext(nc) as tc:
        body(tc, aps)
    nc.compile()
    return nc


def body(tc, aps):
    nc = tc.nc
    x, xa = aps["x"], aps["xa"]
    Wq, Wk, Wv, Wo = aps["Wq"], aps["Wk"], aps["Wv"], aps["Wo"]
    bq, bv, bo = aps["bq"], aps["bv"], aps["bo"]
    out, qk = aps["out"], aps["qk"]

    with (
        tc.tile_pool(name="const", bufs=1) as const,
        tc.tile_pool(name="xTp", bufs=1) as xTp,
        tc.tile_pool(name="xaTp", bufs=1) as xaTp,
        tc.tile_pool(name="wvTp", bufs=1) as wvTp,
    ):
        # ---- constants
        ident_f = const.tile([128, 128], F32)
        make_identity(nc, ident_f)
        ident = const.tile([128, 128], F32R)
        nc.vector.tensor_copy(ident, ident_f)
        bq_sb = const.tile([128, ND], F32)
        nc.sync.dma_start(bq_sb, bq.rearrange("(j p) -> p j", p=128))
        bvbc = const.tile([128, D], F32)
        nc.sync.dma_start(bvbc, bass.AP(tensor=bv.tensor, offset=0,
                                        ap=[[0, 128], [1, D]]))
        bobc = const.tile([128, D], F32)
        nc.sync.dma_start(bobc, bass.AP(tensor=bo.tensor, offset=0,
                                        ap=[[0, 128], [1, D]]))
        vones = const.tile([128, NST * 4], F32)
        nc.vector.memset(vones, 1.0)

        xT = xTp.tile([128, ND, T], F32R)     # xT[p, kk, t] = x[t, kk*128+p]
        xaT = xaTp.tile([128, ND, S], F32R)   # xaT[p, kk, s] = xa[s, kk*128+p]
        wvT = wvTp.tile([128, ND, T], F32R)   # wvT[p, j, t] = attnout[t, j*128+p]

        # ---- phase 0: load + PE-transpose x and xa (PSUM -> SBUF via DMA)
        with (
            tc.tile_pool(name="nat", bufs=3) as nat,
            tc.tile_pool(name="tps", bufs=8, space="PSUM") as tps,
        ):
            # x: one 448-wide pass; 8 psum tiles (one per d-tile kk)
            xins = []
            for ti in range(NTT):
                xin = nat.tile([128, D], F32R, tag="natx", bufs=NTT)
                nc.sync.dma_start(xin[:TT], x[ti * TT:(ti + 1) * TT, :])
                xins.append(xin)
            for kk in range(ND):
                pt = tps.tile([128, T], F32R, tag="tp")
                for ti in range(NTT):
                    nc.tensor.transpose(
                        pt[:, ti * TT:(ti + 1) * TT],
                        xins[ti][:TT, kk * 128:(kk + 1) * 128],
                        ident[:TT, :TT])
                nc.scalar.copy(xT[:, kk, :], pt)
            # xa: three 500-wide passes (5 s-subtiles of 100 each —
            # fp32r PE-transpose needs an even partition count)
            XAL = 100
            for c in range(NSC):
                xains = []
                for u in range(5):
                    xain = nat.tile([128, D], F32R, tag="natxa", bufs=6)
                    r0 = c * SC + u * XAL
                    nc.sync.dma_start(xain[:XAL], xa[r0:r0 + XAL, :])
                    xains.append(xain)
                for kk in range(ND):
                    pt = tps.tile([128, SC], F32R, tag="tp")
                    for u in range(5):
                        nc.tensor.transpose(
                            pt[:, u * XAL:(u + 1) * XAL],
                            xains[u][:XAL, kk * 128:(kk + 1) * 128],
                            ident[:XAL, :XAL])
                    nc.scalar.copy(xaT[:, kk, c * SC:(c + 1) * SC], pt)

        # ---- main loop over head pairs
        with (
            tc.tile_pool(name="wqp", bufs=2) as wqp,
            tc.tile_pool(name="wkp", bufs=2) as wkp,
            tc.tile_pool(name="wvp", bufs=2) as wvp,
            tc.tile_pool(name="qtp", bufs=2) as qtp,
            tc.tile_pool(name="ktp", bufs=2) as ktp,
            tc.tile_pool(name="vgp", bufs=2) as vgp,
            tc.tile_pool(name="etp", bufs=3) as etp,
            tc.tile_pool(name="nrm", bufs=4) as nrm,
            tc.tile_pool(name="pj", bufs=2, space="PSUM") as pj,
            tc.tile_pool(name="pqk", bufs=2, space="PSUM") as pqk,
            tc.tile_pool(name="pqt", bufs=2, space="PSUM") as pqt,
            tc.tile_pool(name="pu", bufs=2, space="PSUM") as pu,
        ):
            vg = None
            for j in range(ND):  # head pair j -> heads 2j, 2j+1
                if j % 2 == 0:
                    # ---- v for 4-head group g = j//2 (d cols 256g..256g+256)
                    g = j // 2
                    wv_t = wvp.tile([128, ND, 256], F32R, tag="wv")
                    nc.sync.dma_start(
                        wv_t, Wv[:, 256 * g:256 * (g + 1)]
                        .rearrange("(kk p) n -> p kk n", p=128))
                    vg = vgp.tile([128, NST, 4, 65], F32R, tag="vg")
                    nc.vector.tensor_copy(vg[:, :, :, 64:65], vones)
                    for st in range(NST):
                        pv = pj.tile([128, 512], F32, tag="pj")
                        for kk in range(ND):
                            nc.tensor.matmul(
                                pv[:ST, :256],
                                xaT[:, kk, st * ST:(st + 1) * ST],
                                wv_t[:, kk, :],
                                start=(kk == 0), stop=(kk == ND - 1))
                        nc.vector.tensor_add(
                            vg[:ST, st, :, 0:64],
                            pv[:ST, :256],
                            bvbc[:ST, 256 * g:256 * (g + 1)])

                # ---- qT_j, kT_j (column slice j of Wq/Wk)
                wq_t = wqp.tile([128, ND, 128], F32R, tag="wq")
                nc.sync.dma_start(
                    wq_t, Wq[:, 128 * j:128 * (j + 1)]
                    .rearrange("(kk p) n -> p kk n", p=128))
                wk_t = wkp.tile([128, ND, 128], F32R, tag="wk")
                nc.sync.dma_start(
                    wk_t, Wk[:, 128 * j:128 * (j + 1)]
                    .rearrange("(kk p) n -> p kk n", p=128))

                qt = qtp.tile([128, T], F32R, tag="qt")
                pq = pj.tile([128, 512], F32, tag="pj")
                for kk in range(ND):
                    nc.tensor.matmul(pq[:, :T], wq_t[:, kk, :], xT[:, kk, :],
                                     start=(kk == 0), stop=(kk == ND - 1))
                nc.vector.tensor_scalar_add(qt, pq[:, :T], bq_sb[:, j:j + 1])

                kt = ktp.tile([128, S], F32R, tag="kt")
                for c in range(NSC):
                    pk = pj.tile([128, 512], F32, tag="pj")
                    for kk in range(ND):
                        nc.tensor.matmul(
                            pk[:, :SC],
                            wk_t[:, kk, :],
                            xaT[:, kk, c * SC:(c + 1) * SC],
                            start=(kk == 0), stop=(kk == ND - 1))
                    nc.vector.tensor_scalar_mul(
                        kt[:, c * SC:(c + 1) * SC], pk[:, :SC], KSCALE)

                # ---- attention for the two heads of pair j
                for hh in range(2):
                    h = 2 * j + hh
                    base = 64 * hh
                    qh = qt[base:base + 64, :]
                    kh = kt[base:base + 64, :]

                    # (a) qk[t,s] -> PSUM -> SBUF copy -> HBM
                    for ti in range(NTT):
                        for c in range(NSC):
                            pqk_t = pqk.tile([128, SC], F32, tag="pqk")
                            nc.tensor.matmul(
                                pqk_t[:TT],
                                qh[:, ti * TT:(ti + 1) * TT],
                                kh[:, c * SC:(c + 1) * SC])
                            qksb = etp.tile([128, SC], F32, tag="qksb",
                                            bufs=4)
                            if (ti * NSC + c) % 2 == 0:
                                nc.vector.tensor_copy(qksb[:TT], pqk_t[:TT])
                            else:
                                nc.scalar.copy(qksb[:TT], pqk_t[:TT])
                            nc.sync.dma_start(
                                qk[h, ti * TT:(ti + 1) * TT,
                                   c * SC:(c + 1) * SC],
                                qksb[:TT])

                    # (b) qkT[s,t] -> exp -> eT ; U^T accumulation
                    pu_t = pu.tile([128, T], F32, tag="pu")
                    for st in range(NST):
                        pqt_t = pqt.tile([128, T], F32, tag="pqt")
                        nc.tensor.matmul(
                            pqt_t[:ST],
                            kh[:, st * ST:(st + 1) * ST],
                            qh)
                        et = etp.tile([128, T], F32R, tag="et")
                        nc.scalar.activation(et[:ST], pqt_t[:ST], AF.Exp)
                        nc.tensor.matmul(
                            pu_t[:65],
                            vg[:ST, st, 2 * (j % 2) + hh, :],
                            et[:ST],
                            start=(st == 0), stop=(st == NST - 1))

                    # normalize: wvT rows = U / Z
                    rz = nrm.tile([128, T], F32, tag="rz", bufs=2)
                    nc.vector.reciprocal(rz[0:1], pu_t[64:65])
                    zbc = nrm.tile([128, T], F32, tag="zbc", bufs=2)
                    nc.gpsimd.partition_broadcast(zbc[:64], rz[0:1])
                    nc.vector.tensor_mul(
                        wvT[base:base + 64, j, :], pu_t[0:64], zbc[:64])

        # ---- output projection: out = wvT.T @ Wo + bo
        with (
            tc.tile_pool(name="wop", bufs=2) as wop,
            tc.tile_pool(name="osb", bufs=3) as osb,
            tc.tile_pool(name="po", bufs=2, space="PSUM") as po,
        ):
            for half in range(2):
                wo_t = wop.tile([128, ND, 512], F32R, tag="wo")
                nc.sync.dma_start(
                    wo_t, Wo[:, 512 * half:512 * (half + 1)]
                    .rearrange("(kk p) n -> p kk n", p=128))
                for ti in range(NTT):
                    pot = po.tile([128, 512], F32, tag="po")
                    for kk in range(ND):
                        nc.tensor.matmul(
                            pot[:TT],
                            wvT[:, kk, ti * TT:(ti + 1) * TT],
                            wo_t[:, kk, :],
                            start=(kk == 0), stop=(kk == ND - 1))
                    ot = osb.tile([128, 512], F32, tag="ot")
                    nc.vector.tensor_add(ot[:TT], pot[:TT],
                                         bobc[:TT, 512 * half:512 * (half + 1)])
                    nc.sync.dma_start(
                        out[ti * TT:(ti + 1) * TT,
                            512 * half:512 * (half + 1)],
                        ot[:TT])


_NC_CACHE = {}


def _get_nc():
    if "nc" not in _NC_CACHE:
        _NC_CACHE["nc"] = build()
    return _NC_CACHE["nc"]


def _run(in_maps, **kw):
    return bass_utils.run_bass_kernel_spmd(
        _get_nc(), in_maps, core_ids=list(range(B)), **kw)


def _make_in_maps(x, xa, Wq, bq, Wk, Wv, bv, Wo, bo):
    f = lambda a: np.ascontiguousarray(np.asarray(a, dtype=np.float32))
    x, xa = f(x), f(xa)
    shared = {"Wq": f(Wq), "Wk": f(Wk), "Wv": f(Wv), "Wo": f(Wo),
              "bq": f(bq), "bv": f(bv), "bo": f(bo)}
    return [dict(x=x[b], xa=xa[b], **shared) for b in range(B)]


def kernel(x, xa, Wq, bq, Wk, Wv, bv, Wo, bo):
    in_maps = _make_in_maps(x, xa, Wq, bq, Wk, Wv, bv, Wo, bo)
    res = _run(in_maps)
    out = np.stack([r["out"] for r in res.results])
    qk = np.stack([r["qk"] for r in res.results])
    return out, qk


# revision 18
# speedup vs baseline: 1.1185x; 1.1185x over previous
"""MultiHeadCrossAttention on 8 trn2 NeuronCores.

Strategy: pure batch data-parallelism — B=8 batches, one per core, zero
collectives. Each core computes its batch's full cross-attention:

  q = x @ Wq + bq ; k = xa @ Wk ; v = xa @ Wv + bv
  qk[h,t,s] = (q*sc)·(k*sc)   (sc = Dh**-0.25; folded as k *= 1/8 exactly)
  out = softmax(qk) @ v @ Wo + bo

Per-core kernel layout (PE matmul computes out[M,N] = lhsT[K,M].T @ rhs[K,N],
contraction K on partitions):
  - x/xa are PE-transposed once into xT[d,t] / xaT[d,s] (PSUM->SBUF via DMA).
  - per head-pair j: qT[d,t], kT[d,s] column slices; v[s,d] per 4-head group.
  - qk is computed twice: once as [t,s] (streamed straight from PSUM to the
    HBM qk output), once as [s,t] feeding ACT exp -> eT.
  - U^T = [v | 1]^T @ eT gives the unnormalized head output and the softmax
    denominator Z (row 64) in one PSUM accumulation.
  - normalize with 1/Z partition-broadcast, then out = wvT.T @ Wo + bo.

All matmuls run in float32r (TF32-like; 1 cyc/row at N>=256; ~1e-4 rel err).
"""
import sys

import numpy as np

try:
    import concourse.bacc as bacc
except ImportError:  # fresh grading dir: make sure the repo root is importable
    for p in ("/opt/trn_rl_repo", "/root/.axon_site/_ro/trn_rl_repo"):
        if p not in sys.path:
            sys.path.append(p)
    import concourse.bacc as bacc

import concourse.bass as bass
import concourse.mybir as mybir
import concourse.tile as tile
from concourse import bass_utils
from concourse.masks import make_identity

F32 = mybir.dt.float32
F32R = mybir.dt.float32r
AF = mybir.ActivationFunctionType

B, T, S, D, H, Dh = 8, 448, 1500, 1024, 16, 64
ND = D // 128          # 8 d-tiles / k-tiles of 128
TT, NTT = 112, 4       # t tiling: 4 x 112
ST, NST = 125, 12      # s tiling for qkT / v: 12 x 125
SC, NSC = 500, 3       # s chunks for qk(a) / kT: 3 x 500 (4 x 125 each)
KSCALE = 0.125         # (Dh**-0.25)**2 = 1/8, exact in fp


def build():
    nc = bacc.Bacc("TRN2", target_bir_lowering=False, debug=False)
    aps = {}
    for name, shape, dt in [
        ("x", [T, D], F32R), ("xa", [S, D], F32R),
        ("Wq", [D, D], F32R), ("Wk", [D, D], F32R),
        ("Wv", [D, D], F32R), ("Wo", [D, D], F32R),
        ("bq", [D], F32), ("bv", [D], F32), ("bo", [D], F32),
    ]:
        aps[name] = nc.dram_tensor(name, shape, dt, kind="ExternalInput").ap()
    aps["out"] = nc.dram_tensor("out", [T, D], F32, kind="ExternalOutput").ap()
    aps["qk"] = nc.dram_tensor("qk", [H, T, S], F32, kind="ExternalOutput").ap()

    with tile.TileContext(nc) as tc:
        body(tc, aps)
    nc.compile()
    return nc


def body(tc, aps):
    nc = tc.nc
    x, xa = aps["x"], aps["xa"]
    Wq, Wk, Wv, Wo = aps["Wq"], aps["Wk"], aps["Wv"], aps["Wo"]
    bq, bv, bo = aps["bq"], aps["bv"], aps["bo"]
    out, qk = aps["out"], aps["qk"]

    with (
        tc.tile_pool(name="const", bufs=1) as const,
        tc.tile_pool(name="xTp", bufs=1) as xTp,
        tc.tile_pool(name="xaTp", bufs=1) as xaTp,
        tc.tile_pool(name="wvTp", bufs=1) as wvTp,
    ):
        # ---- constants
        ident_f = const.tile([128, 128], F32)
        make_identity(nc, ident_f)
        ident = const.tile([128, 128], F32R)
        nc.vector.tensor_copy(ident, ident_f)
        bq_sb = const.tile([128, ND], F32)
        nc.sync.dma_start(bq_sb, bq.rearrange("(j p) -> p j", p=128))
        bvbc = const.tile([128, D], F32)
        nc.sync.dma_start(bvbc, bass.AP(tensor=bv.tensor, offset=0,
                                        ap=[[0, 128], [1, D]]))
        bobc = const.tile([128, D], F32)
        nc.sync.dma_start(bobc, bass.AP(tensor=bo.tensor, offset=0,
                                        ap=[[0, 128], [1, D]]))
        vones = const.tile([128, NST * 4], F32)
        nc.vector.memset(vones, 1.0)

        xT = xTp.tile([128, ND, T], F32R)     # xT[p, kk, t] = x[t, kk*128+p]
        xaT = xaTp.tile([128, ND, S], F32R)   # xaT[p, kk, s] = xa[s, kk*128+p]
        wvT = wvTp.tile([128, ND, T], F32R)   # wvT[p, j, t] = attnout[t, j*128+p]

        # ---- phase 0: load + PE-transpose x and xa (PSUM -> SBUF via DMA)
        with (
            tc.tile_pool(name="nat", bufs=3) as nat,
            tc.tile_pool(name="tps", bufs=8, space="PSUM") as tps,
        ):
            # x: one 448-wide pass; 8 psum tiles (one per d-tile kk)
            xins = []
            for ti in range(NTT):
                xin = nat.tile([128, D], F32R, tag="natx", bufs=NTT)
                nc.sync.dma_start(xin[:TT], x[ti * TT:(ti + 1) * TT, :])
                xins.append(xin)
            for kk in range(ND):
                pt = tps.tile([128, T], F32R, tag="tp")
                for ti in range(NTT):
                    nc.tensor.transpose(
                        pt[:, ti * TT:(ti + 1) * TT],
                        xins[ti][:TT, kk * 128:(kk + 1) * 128],
                        ident[:TT, :TT])
                nc.scalar.copy(xT[:, kk, :], pt)
            # xa: three 500-wide passes (5 s-subtiles of 100 each —
            # fp32r PE-transpose needs an even partition count)
            XAL = 100
            for c in range(NSC):
                xains = []
                for u in range(5):
                    xain = nat.tile([128, D], F32R, tag="natxa", bufs=6)
                    r0 = c * SC + u * XAL
                    nc.sync.dma_start(xain[:XAL], xa[r0:r0 + XAL, :])
                    xains.append(xain)
                for kk in range(ND):
                    pt = tps.tile([128, SC], F32R, tag="tp")
                    for u in range(5):
                        nc.tensor.transpose(
                            pt[:, u * XAL:(u + 1) * XAL],
                            xains[u][:XAL, kk * 128:(kk + 1) * 128],
                            ident[:XAL, :XAL])
                    nc.scalar.copy(xaT[:, kk, c * SC:(c + 1) * SC], pt)

        # ---- main loop over head pairs
        with (
            tc.tile_pool(name="wqp", bufs=2) as wqp,
            tc.tile_pool(name="wkp", bufs=2) as wkp,
            tc.tile_pool(name="wvp", bufs=2) as wvp,
            tc.tile_pool(name="qtp", bufs=2) as qtp,
            tc.tile_pool(name="ktp", bufs=2) as ktp,
            tc.tile_pool(name="vgp", bufs=2) as vgp,
            tc.tile_pool(name="etp", bufs=3) as etp,
            tc.tile_pool(name="nrm", bufs=4) as nrm,
            tc.tile_pool(name="pbig", bufs=3, space="PSUM") as pbig,
            tc.tile_pool(name="pqt", bufs=3, space="PSUM") as pqt,
            tc.tile_pool(name="pu", bufs=2, space="PSUM") as pu,
        ):
            vgs = [None, None]
            for j in range(ND):  # head pair j -> heads 2j, 2j+1
                if j % 4 == 0:
                    # ---- v for half hf (8 heads, d cols 512hf..512hf+512)
                    hf = j // 4
                    wv_t = wvp.tile([128, ND, 512], F32R, tag="wv")
                    nc.sync.dma_start(
                        wv_t, Wv[:, 512 * hf:512 * (hf + 1)]
                        .rearrange("(kk p) n -> p kk n", p=128))
                    vgs = []
                    for g2 in range(2):
                        vg_t = vgp.tile([128, NST, 4, 65], F32R, tag="vg",
                                        name=f"vg{2 * hf + g2}")
                        nc.vector.tensor_copy(vg_t[:, :, :, 64:65], vones)
                        vgs.append(vg_t)
                    for st in range(NST):
                        pv = pbig.tile([128, 512], F32, tag="pbig")
                        for kk in range(ND):
                            nc.tensor.matmul(
                                pv[:ST],
                                xaT[:, kk, st * ST:(st + 1) * ST],
                                wv_t[:, kk, :],
                                start=(kk == 0), stop=(kk == ND - 1))
                        for g2 in range(2):
                            nc.vector.tensor_add(
                                vgs[g2][:ST, st, :, 0:64],
                                pv[:ST, 256 * g2:256 * (g2 + 1)],
                                bvbc[:ST, 512 * hf + 256 * g2:
                                     512 * hf + 256 * (g2 + 1)])

                # ---- qT_j, kT_j (column slice j of Wq/Wk)
                wq_t = wqp.tile([128, ND, 128], F32R, tag="wq")
                nc.sync.dma_start(
                    wq_t, Wq[:, 128 * j:128 * (j + 1)]
                    .rearrange("(kk p) n -> p kk n", p=128))
                wk_t = wkp.tile([128, ND, 128], F32R, tag="wk")
                nc.sync.dma_start(
                    wk_t, Wk[:, 128 * j:128 * (j + 1)]
                    .rearrange("(kk p) n -> p kk n", p=128))

                qt = qtp.tile([128, T], F32R, tag="qt")
                pq = pbig.tile([128, 512], F32, tag="pbig")
                for kk in range(ND):
                    nc.tensor.matmul(pq[:, :T], wq_t[:, kk, :], xT[:, kk, :],
                                     start=(kk == 0), stop=(kk == ND - 1))
                nc.vector.tensor_scalar_add(qt, pq[:, :T], bq_sb[:, j:j + 1])

                kt = ktp.tile([128, S], F32R, tag="kt")
                for c in range(NSC):
                    pk = pbig.tile([128, 512], F32, tag="pbig")
                    for kk in range(ND):
                        nc.tensor.matmul(
                            pk[:, :SC],
                            wk_t[:, kk, :],
                            xaT[:, kk, c * SC:(c + 1) * SC],
                            start=(kk == 0), stop=(kk == ND - 1))
                    nc.vector.tensor_scalar_mul(
                        kt[:, c * SC:(c + 1) * SC], pk[:, :SC], KSCALE)

                # ---- attention for the two heads of pair j
                # NOTE: fp32r matmuls cannot be row-group packed (HW
                # corruption verified) — heads run sequentially.
                vg = vgs[(j % 4) // 2]
                for hh in range(2):
                    h = 2 * j + hh
                    base = 64 * hh
                    qh = qt[base:base + 64, :]
                    kh = kt[base:base + 64, :]

                    # (a) qk[t,s] -> PSUM -> SBUF copy -> HBM
                    for ti in range(NTT):
                        for c in range(NSC):
                            pqk_t = pbig.tile([128, SC], F32, tag="pbig")
                            nc.tensor.matmul(
                                pqk_t[:TT],
                                qh[:, ti * TT:(ti + 1) * TT],
                                kh[:, c * SC:(c + 1) * SC])
                            qksb = etp.tile([128, SC], F32, tag="qksb",
                                            bufs=4)
                            if (ti * NSC + c) % 2 == 0:
                                nc.vector.tensor_copy(qksb[:TT], pqk_t[:TT])
                            else:
                                nc.scalar.copy(qksb[:TT], pqk_t[:TT])
                            nc.sync.dma_start(
                                qk[h, ti * TT:(ti + 1) * TT,
                                   c * SC:(c + 1) * SC],
                                qksb[:TT])

                    # (b) qkT[s,t] -> exp -> eT ; U^T accumulation
                    pu_t = pu.tile([128, T], F32, tag="pu")
                    for st in range(NST):
                        pqt_t = pqt.tile([128, T], F32, tag="pqt")
                        nc.tensor.matmul(
                            pqt_t[:ST],
                            kh[:, st * ST:(st + 1) * ST],
                            qh)
                        et = etp.tile([128, T], F32R, tag="et")
                        nc.scalar.activation(et[:ST], pqt_t[:ST], AF.Exp)
                        nc.tensor.matmul(
                            pu_t[:65],
                            vg[:ST, st, 2 * (j % 2) + hh, :],
                            et[:ST],
                            start=(st == 0), stop=(st == NST - 1))

                    # normalize: wvT rows = U * (1/Z)
                    rz = nrm.tile([128, T], F32, tag="rz", bufs=2)
                    nc.vector.tensor_copy(rz[0:1], pu_t[64:65])
                    rzi = nrm.tile([128, T], F32, tag="rzi", bufs=2)
                    nc.vector.reciprocal_approx_fast(rzi[0:1], rz[0:1])
                    zbc = nrm.tile([128, T], F32, tag="zbc", bufs=2)
                    nc.gpsimd.partition_broadcast(zbc[:64], rzi[0:1])
                    nc.vector.tensor_mul(
                        wvT[base:base + 64, j, :], pu_t[0:64], zbc[:64])

        # ---- output projection: out = wvT.T @ Wo + bo
        with (
            tc.tile_pool(name="wop", bufs=2) as wop,
            tc.tile_pool(name="osb", bufs=3) as osb,
            tc.tile_pool(name="po", bufs=2, space="PSUM") as po,
        ):
            for half in range(2):
                wo_t = wop.tile([128, ND, 512], F32R, tag="wo")
                nc.sync.dma_start(
                    wo_t, Wo[:, 512 * half:512 * (half + 1)]
                    .rearrange("(kk p) n -> p kk n", p=128))
                for ti in range(NTT):
                    pot = po.tile([128, 512], F32, tag="po")
                    for kk in range(ND):
                        nc.tensor.matmul(
                            pot[:TT],
                            wvT[:, kk, ti * TT:(ti + 1) * TT],
                            wo_t[:, kk, :],
                            start=(kk == 0), stop=(kk == ND - 1))
                    ot = osb.tile([128, 512], F32, tag="ot")
                    nc.vector.tensor_add(ot[:TT], pot[:TT],
                                         bobc[:TT, 512 * half:512 * (half + 1)])
                    nc.sync.dma_start(
                        out[ti * TT:(ti + 1) * TT,
                            512 * half:512 * (half + 1)],
                        ot[:TT])


_NC_CACHE = {}


def _get_nc():
    if "nc" not in _NC_CACHE:
        _NC_CACHE["nc"] = build()
    return _NC_CACHE["nc"]


def _run(in_maps, **kw):
    return bass_utils.run_bass_kernel_spmd(
        _get_nc(), in_maps, core_ids=list(range(B)), **kw)


def _make_in_maps(x, xa, Wq, bq, Wk, Wv, bv, Wo, bo):
    f = lambda a: np.ascontiguousarray(np.asarray(a, dtype=np.float32))
    x, xa = f(x), f(xa)
    shared = {"Wq": f(Wq), "Wk": f(Wk), "Wv": f(Wv), "Wo": f(Wo),
              "bq": f(bq), "bv": f(bv), "bo": f(bo)}
    return [dict(x=x[b], xa=xa[b], **shared) for b in range(B)]


def kernel(x, xa, Wq, bq, Wk, Wv, bv, Wo, bo):
    in_maps = _make_in_maps(x, xa, Wq, bq, Wk, Wv, bv, Wo, bo)
    res = _run(in_maps)
    out = np.stack([r["out"] for r in res.results])
    qk = np.stack([r["qk"] for r in res.results])
    return out, qk


# revision 23
# speedup vs baseline: 1.1598x; 1.0369x over previous
"""MultiHeadCrossAttention on 8 trn2 NeuronCores.

Strategy: pure batch data-parallelism — B=8 batches, one per core, zero
collectives. Each core computes its batch's full cross-attention:

  q = x @ Wq + bq ; k = xa @ Wk ; v = xa @ Wv + bv
  qk[h,t,s] = (q*sc)·(k*sc)   (sc = Dh**-0.25; folded as k *= 1/8 exactly)
  out = softmax(qk) @ v @ Wo + bo

Per-core kernel layout (PE matmul computes out[M,N] = lhsT[K,M].T @ rhs[K,N],
contraction K on partitions):
  - x/xa are PE-transposed once into xT[d,t] / xaT[d,s] (PSUM->SBUF via DMA).
  - per head-pair j: qT[d,t], kT[d,s] column slices; v[s,d] per 4-head group.
  - qk is computed twice: once as [t,s] (streamed straight from PSUM to the
    HBM qk output), once as [s,t] feeding ACT exp -> eT.
  - U^T = [v | 1]^T @ eT gives the unnormalized head output and the softmax
    denominator Z (row 64) in one PSUM accumulation.
  - normalize with 1/Z partition-broadcast, then out = wvT.T @ Wo + bo.

All matmuls run in float32r (TF32-like; 1 cyc/row at N>=256; ~1e-4 rel err).
"""
import sys

import numpy as np

try:
    import concourse.bacc as bacc
except ImportError:  # fresh grading dir: make sure the repo root is importable
    for p in ("/opt/trn_rl_repo", "/root/.axon_site/_ro/trn_rl_repo"):
        if p not in sys.path:
            sys.path.append(p)
    import concourse.bacc as bacc

import concourse.bass as bass
import concourse.mybir as mybir
import concourse.tile as tile
from concourse import bass_utils
from concourse.masks import make_identity

F32 = mybir.dt.float32
F32R = mybir.dt.float32r
AF = mybir.ActivationFunctionType

B, T, S, D, H, Dh = 8, 448, 1500, 1024, 16, 64
ND = D // 128          # 8 d-tiles / k-tiles of 128
TT, NTT = 112, 4       # t tiling: 4 x 112
ST, NST = 125, 12      # s tiling for qkT / v: 12 x 125
SC, NSC = 500, 3       # s chunks for qk(a) / kT: 3 x 500 (4 x 125 each)
KSCALE = 0.125         # (Dh**-0.25)**2 = 1/8, exact in fp


def build():
    nc = bacc.Bacc("TRN2", target_bir_lowering=False, debug=False)
    aps = {}
    for name, shape, dt in [
        ("x", [T, D], F32R), ("xa", [S, D], F32R),
        ("Wq", [D, D], F32R), ("Wk", [D, D], F32R),
        ("Wv", [D, D], F32R), ("Wo", [D, D], F32R),
        ("bq", [D], F32), ("bv", [D], F32), ("bo", [D], F32),
    ]:
        aps[name] = nc.dram_tensor(name, shape, dt, kind="ExternalInput").ap()
    aps["out"] = nc.dram_tensor("out", [T, D], F32, kind="ExternalOutput").ap()
    aps["qk"] = nc.dram_tensor("qk", [H, T, S], F32, kind="ExternalOutput").ap()

    with tile.TileContext(nc) as tc:
        body(tc, aps)
    nc.compile()
    return nc


def body(tc, aps):
    nc = tc.nc
    x, xa = aps["x"], aps["xa"]
    Wq, Wk, Wv, Wo = aps["Wq"], aps["Wk"], aps["Wv"], aps["Wo"]
    bq, bv, bo = aps["bq"], aps["bv"], aps["bo"]
    out, qk = aps["out"], aps["qk"]

    with (
        tc.tile_pool(name="const", bufs=1) as const,
        tc.tile_pool(name="xTp", bufs=1) as xTp,
        tc.tile_pool(name="xaTp", bufs=1) as xaTp,
        tc.tile_pool(name="wvTp", bufs=1) as wvTp,
    ):
        # ---- constants
        ident_f = const.tile([128, 128], F32)
        make_identity(nc, ident_f)
        ident = const.tile([128, 128], F32R)
        nc.vector.tensor_copy(ident, ident_f)
        bq_sb = const.tile([128, ND], F32)
        nc.sync.dma_start(bq_sb, bq.rearrange("(j p) -> p j", p=128))
        bvbc = const.tile([128, D], F32)
        nc.sync.dma_start(bvbc, bass.AP(tensor=bv.tensor, offset=0,
                                        ap=[[0, 128], [1, D]]))
        bobc = const.tile([128, D], F32)
        nc.sync.dma_start(bobc, bass.AP(tensor=bo.tensor, offset=0,
                                        ap=[[0, 128], [1, D]]))
        vones = const.tile([128, NST * 4], F32)
        nc.vector.memset(vones, 1.0)
        # HAM heater operands: tiny bf16 matmuls sprinkled into the PE
        # stream keep the activity monitor at K=8/8 (fp32r matmuls alone
        # leave it oscillating at half clock).
        hA = const.tile([128, 1], mybir.dt.bfloat16)
        nc.vector.memset(hA, 1.0)
        hB = const.tile([128, 128], mybir.dt.bfloat16)
        nc.vector.memset(hB, 0.0)

        xT = xTp.tile([128, ND, T], F32R)     # xT[p, kk, t] = x[t, kk*128+p]
        xaT = xaTp.tile([128, ND, S], F32R)   # xaT[p, kk, s] = xa[s, kk*128+p]
        wvT = wvTp.tile([128, ND, T], F32R)   # wvT[p, j, t] = attnout[t, j*128+p]

        # ---- phase 0: load + PE-transpose x and xa (PSUM -> SBUF via DMA)
        with (
            tc.tile_pool(name="nat", bufs=3) as nat,
            tc.tile_pool(name="tps", bufs=8, space="PSUM") as tps,
        ):
            # x: one 448-wide pass; 8 psum tiles (one per d-tile kk)
            xins = []
            for ti in range(NTT):
                xin = nat.tile([128, D], F32R, tag="natx", bufs=NTT)
                nc.sync.dma_start(xin[:TT], x[ti * TT:(ti + 1) * TT, :])
                xins.append(xin)
            for kk in range(ND):
                pt = tps.tile([128, T], F32R, tag="tp")
                for ti in range(NTT):
                    nc.tensor.transpose(
                        pt[:, ti * TT:(ti + 1) * TT],
                        xins[ti][:TT, kk * 128:(kk + 1) * 128],
                        ident[:TT, :TT])
                nc.scalar.copy(xT[:, kk, :], pt)
            # xa: three 500-wide passes (5 s-subtiles of 100 each —
            # fp32r PE-transpose needs an even partition count)
            XAL = 100
            for c in range(NSC):
                xains = []
                for u in range(5):
                    xain = nat.tile([128, D], F32R, tag="natxa", bufs=6)
                    r0 = c * SC + u * XAL
                    nc.sync.dma_start(xain[:XAL], xa[r0:r0 + XAL, :])
                    xains.append(xain)
                for kk in range(ND):
                    pt = tps.tile([128, SC], F32R, tag="tp")
                    for u in range(5):
                        nc.tensor.transpose(
                            pt[:, u * XAL:(u + 1) * XAL],
                            xains[u][:XAL, kk * 128:(kk + 1) * 128],
                            ident[:XAL, :XAL])
                    nc.scalar.copy(xaT[:, kk, c * SC:(c + 1) * SC], pt)

        # ---- main loop over head pairs
        with (
            tc.tile_pool(name="wqp", bufs=2) as wqp,
            tc.tile_pool(name="wkp", bufs=2) as wkp,
            tc.tile_pool(name="wvp", bufs=2) as wvp,
            tc.tile_pool(name="qtp", bufs=2) as qtp,
            tc.tile_pool(name="ktp", bufs=2) as ktp,
            tc.tile_pool(name="vgp", bufs=2) as vgp,
            tc.tile_pool(name="etp", bufs=3) as etp,
            tc.tile_pool(name="nrm", bufs=4) as nrm,
            tc.tile_pool(name="pbig", bufs=2, space="PSUM") as pbig,
            tc.tile_pool(name="pqt", bufs=3, space="PSUM") as pqt,
            tc.tile_pool(name="pu", bufs=2, space="PSUM") as pu,
            tc.tile_pool(name="phv", bufs=1, space="PSUM") as phv,
        ):
            heat_ps = phv.tile([128, 128], F32, tag="heat")

            def heat():
                nc.tensor.matmul(heat_ps[:1], hA, hB, skip_group_check=True)

            vgs = [None, None]
            for j in range(ND):  # head pair j -> heads 2j, 2j+1
                # ---- qT_j, kT_j (column slice j of Wq/Wk)
                wq_t = wqp.tile([128, ND, 128], F32R, tag="wq")
                nc.sync.dma_start(
                    wq_t, Wq[:, 128 * j:128 * (j + 1)]
                    .rearrange("(kk p) n -> p kk n", p=128))
                wk_t = wkp.tile([128, ND, 128], F32R, tag="wk")
                nc.sync.dma_start(
                    wk_t, Wk[:, 128 * j:128 * (j + 1)]
                    .rearrange("(kk p) n -> p kk n", p=128))

                qt = qtp.tile([128, T], F32R, tag="qt")
                pq = pbig.tile([128, 512], F32, tag="pbig")
                for kk in range(ND):
                    nc.tensor.matmul(pq[:, :T], wq_t[:, kk, :], xT[:, kk, :],
                                     start=(kk == 0), stop=(kk == ND - 1))
                nc.vector.tensor_scalar_add(qt, pq[:, :T], bq_sb[:, j:j + 1])
                heat()

                kt = ktp.tile([128, S], F32R, tag="kt")
                for c in range(NSC):
                    pk = pbig.tile([128, 512], F32, tag="pbig")
                    for kk in range(ND):
                        nc.tensor.matmul(
                            pk[:, :SC],
                            wk_t[:, kk, :],
                            xaT[:, kk, c * SC:(c + 1) * SC],
                            start=(kk == 0), stop=(kk == ND - 1))
                    nc.vector.tensor_scalar_mul(
                        kt[:, c * SC:(c + 1) * SC], pk[:, :SC], KSCALE)
                    heat()

                if j % 4 == 0:
                    # ---- v for half hf (8 heads, d cols 512hf..512hf+512)
                    hf = j // 4
                    wv_t = wvp.tile([128, ND, 512], F32R, tag="wv")
                    nc.sync.dma_start(
                        wv_t, Wv[:, 512 * hf:512 * (hf + 1)]
                        .rearrange("(kk p) n -> p kk n", p=128))
                    vgs = []
                    for g2 in range(2):
                        vg_t = vgp.tile([128, NST, 4, 65], F32R, tag="vg",
                                        name=f"vg{2 * hf + g2}")
                        nc.vector.tensor_copy(vg_t[:, :, :, 64:65], vones)
                        vgs.append(vg_t)
                    for st in range(NST):
                        pv = pbig.tile([128, 512], F32, tag="pbig")
                        for kk in range(ND):
                            nc.tensor.matmul(
                                pv[:ST],
                                xaT[:, kk, st * ST:(st + 1) * ST],
                                wv_t[:, kk, :],
                                start=(kk == 0), stop=(kk == ND - 1))
                        for g2 in range(2):
                            nc.vector.tensor_add(
                                vgs[g2][:ST, st, :, 0:64],
                                pv[:ST, 256 * g2:256 * (g2 + 1)],
                                bvbc[:ST, 512 * hf + 256 * g2:
                                     512 * hf + 256 * (g2 + 1)])
                        if st % 3 == 2:
                            heat()

                # ---- attention for the two heads of pair j
                # NOTE: fp32r matmuls cannot be row-group packed (HW
                # corruption verified) — heads run sequentially.
                vg = vgs[(j % 4) // 2]
                for hh in range(2):
                    h = 2 * j + hh
                    base = 64 * hh
                    qh = qt[base:base + 64, :]
                    kh = kt[base:base + 64, :]

                    # (a) qk[t,s] -> PSUM -> SBUF copy -> HBM
                    for ti in range(NTT):
                        for c in range(NSC):
                            pqk_t = pbig.tile([128, SC], F32, tag="pbig")
                            nc.tensor.matmul(
                                pqk_t[:TT],
                                qh[:, ti * TT:(ti + 1) * TT],
                                kh[:, c * SC:(c + 1) * SC])
                            qksb = etp.tile([128, SC], F32, tag="qksb",
                                            bufs=4)
                            if (ti * NSC + c) % 2 == 0:
                                nc.vector.tensor_copy(qksb[:TT], pqk_t[:TT])
                            else:
                                nc.scalar.copy(qksb[:TT], pqk_t[:TT])
                            nc.sync.dma_start(
                                qk[h, ti * TT:(ti + 1) * TT,
                                   c * SC:(c + 1) * SC],
                                qksb[:TT])
                            if c == NSC - 1:
                                heat()

                    # (b) qkT[s,t] -> exp -> eT ; U^T accumulation
                    pu_t = pu.tile([128, T], F32, tag="pu")
                    for st in range(NST):
                        pqt_t = pqt.tile([128, T], F32, tag="pqt")
                        nc.tensor.matmul(
                            pqt_t[:ST],
                            kh[:, st * ST:(st + 1) * ST],
                            qh)
                        et = etp.tile([128, T], F32R, tag="et")
                        nc.scalar.activation(et[:ST], pqt_t[:ST], AF.Exp)
                        nc.tensor.matmul(
                            pu_t[:65],
                            vg[:ST, st, 2 * (j % 2) + hh, :],
                            et[:ST],
                            start=(st == 0), stop=(st == NST - 1))
                        if st % 4 == 3:
                            heat()

                    # normalize: wvT rows = U * (1/Z)
                    rz = nrm.tile([128, T], F32, tag="rz", bufs=2)
                    nc.vector.tensor_copy(rz[0:1], pu_t[64:65])
                    rzi = nrm.tile([128, T], F32, tag="rzi", bufs=2)
                    nc.vector.reciprocal_approx_fast(rzi[0:1], rz[0:1])
                    zbc = nrm.tile([128, T], F32, tag="zbc", bufs=2)
                    nc.gpsimd.partition_broadcast(zbc[:64], rzi[0:1])
                    nc.vector.tensor_mul(
                        wvT[base:base + 64, j, :], pu_t[0:64], zbc[:64])

        # ---- output projection: out = wvT.T @ Wo + bo
        with (
            tc.tile_pool(name="wop", bufs=2) as wop,
            tc.tile_pool(name="osb", bufs=3) as osb,
            tc.tile_pool(name="po", bufs=2, space="PSUM") as po,
        ):
            for half in range(2):
                wo_t = wop.tile([128, ND, 512], F32R, tag="wo")
                nc.sync.dma_start(
                    wo_t, Wo[:, 512 * half:512 * (half + 1)]
                    .rearrange("(kk p) n -> p kk n", p=128))
                for ti in range(NTT):
                    pot = po.tile([128, 512], F32, tag="po")
                    for kk in range(ND):
                        nc.tensor.matmul(
                            pot[:TT],
                            wvT[:, kk, ti * TT:(ti + 1) * TT],
                            wo_t[:, kk, :],
                            start=(kk == 0), stop=(kk == ND - 1))
                    ot = osb.tile([128, 512], F32, tag="ot")
                    nc.vector.tensor_add(ot[:TT], pot[:TT],
                                         bobc[:TT, 512 * half:512 * (half + 1)])
                    nc.sync.dma_start(
                        out[ti * TT:(ti + 1) * TT,
                            512 * half:512 * (half + 1)],
                        ot[:TT])


_NC_CACHE = {}


def _get_nc():
    if "nc" not in _NC_CACHE:
        _NC_CACHE["nc"] = build()
    return _NC_CACHE["nc"]


def _run(in_maps, **kw):
    return bass_utils.run_bass_kernel_spmd(
        _get_nc(), in_maps, core_ids=list(range(B)), **kw)


def _make_in_maps(x, xa, Wq, bq, Wk, Wv, bv, Wo, bo):
    f = lambda a: np.ascontiguousarray(np.asarray(a, dtype=np.float32))
    x, xa = f(x), f(xa)
    shared = {"Wq": f(Wq), "Wk": f(Wk), "Wv": f(Wv), "Wo": f(Wo),
              "bq": f(bq), "bv": f(bv), "bo": f(bo)}
    return [dict(x=x[b], xa=xa[b], **shared) for b in range(B)]


def kernel(x, xa, Wq, bq, Wk, Wv, bv, Wo, bo):
    in_maps = _make_in_maps(x, xa, Wq, bq, Wk, Wv, bv, Wo, bo)
    res = _run(in_maps)
    out = np.stack([r["out"] for r in res.results])
    qk = np.stack([r["qk"] for r in res.results])
    return out, qk
